# revision 8
# baseline (speedup 1.0000x reference)
"""Bass/Tile TRN2 kernel for nn_Attention_12489764897521.

attns[b, n] = sum_h W[0, h] * tanh(decoder[b, h] + static[b, h, n] + dynamic[b, h, n])

Full shapes: static/dynamic [32, 256, 10000] f32, decoder [32, 256] f32,
W [1, 256] f32 -> attns [32, 10000] f32.

Sharding: data-parallel over batch B across 8 cores (4 batches/core); W
replicated. The kernel is HBM-bandwidth-bound: 82 MB of input per core at
the ~360-410 GB/s/core HBM share (each NC pair splits one HBM stack), so
the streaming floor is ~200-230 us depending on device tenancy. Everything
else is overhead to hide:

  - prologue (~7 us, fixed): engine boot barriers + ucode table loads; the
    W/decoder preloads ride the gpsimd SWDGE queue so the first packets on
    the two HWDGE rings are big streaming loads.
  - streaming: static loads on the SP HWDGE ring, dynamic on the ACT ring;
    uniform 2500-wide n-chunks, each a fused 3D-AP load of both H-halves
    [128, 2*2500] (2.56 MB/transfer). Pools are 3-deep so the rings never
    wait on compute.
  - per chunk: DVE adds s+d per half -> bf16 tiles; ACT tanh (bf16 in/out,
    2x rate) with the decoder column as bias; PE accumulates the two
    128-long H-half contractions into psum [1, <=500] slices (bf16 matmul,
    1 cyc/row); single-lane psum->SBUF copies alternate DVE/ACT; one store
    per chunk from a double-buffered stage row.
  - tail: the last batch ends 1250, 625, 625 wide so the trailing serial
    add->tanh->matmul->copy->store chain after the final load is short.
"""

from contextlib import ExitStack

import numpy as np

B, H, N = 32, 256, 10000
N_CORES = 8
B_LOC = B // N_CORES  # 4 batches per core
P = 128
NT = H // P  # 2 H-halves
NC = 2500  # n-chunk width; each load fuses both H-halves -> [128, 2*NC]
JC = 500  # matmul free-dim chunk (<= 512, one PSUM bank)

_cache = {}


def _build():
    import concourse.bacc as bacc
    import concourse.mybir as mybir
    import concourse.tile as tile

    nc = bacc.Bacc(
        "TRN2", target_bir_lowering=False, debug=False, num_devices=N_CORES
    )
    st = nc.dram_tensor(
        "static_hidden", [B_LOC, H, N], mybir.dt.float32, kind="ExternalInput"
    ).ap()
    dy = nc.dram_tensor(
        "dynamic_hidden", [B_LOC, H, N], mybir.dt.float32, kind="ExternalInput"
    ).ap()
    dec = nc.dram_tensor(
        "decoder_hidden", [B_LOC, H], mybir.dt.float32, kind="ExternalInput"
    ).ap()
    w = nc.dram_tensor("W", [1, H], mybir.dt.float32, kind="ExternalInput").ap()
    out = nc.dram_tensor(
        "attns", [B_LOC, N], mybir.dt.float32, kind="ExternalOutput"
    ).ap()

    f32 = mybir.dt.float32
    bf16 = mybir.dt.bfloat16
    with tile.TileContext(nc) as tc, ExitStack() as ctx:
        singles = ctx.enter_context(tc.tile_pool(name="singles", bufs=1))
        s_pool = ctx.enter_context(tc.tile_pool(name="s", bufs=3))
        d_pool = ctx.enter_context(tc.tile_pool(name="d", bufs=3))
        a_pool = ctx.enter_context(tc.tile_pool(name="a", bufs=4))
        t_pool = ctx.enter_context(tc.tile_pool(name="t", bufs=6))
        stage_pool = ctx.enter_context(tc.tile_pool(name="stage", bufs=3))
        psum_pool = ctx.enter_context(
            tc.tile_pool(name="psum", bufs=8, space="PSUM")
        )

        # W as two [128, 1] columns (one per H-half), decoder as [128, 1]
        # bias columns indexed [t * B_LOC + b]. These preloads ride the
        # gpsimd SWDGE queue so the two HWDGE rings' first packets are the
        # first big streaming loads.
        w_sb = singles.tile([P, NT], f32)
        w_cols = w.rearrange("o (t p) -> t p o", p=P)
        for t in range(NT):
            nc.gpsimd.dma_start(w_sb[:, t : t + 1], w_cols[t])

        dec_sb = singles.tile([P, NT * B_LOC], f32)
        dec_r = dec.rearrange("b (t p) -> t p b", p=P)
        for t in range(NT):
            nc.gpsimd.dma_start(dec_sb[:, t * B_LOC : (t + 1) * B_LOC], dec_r[t])

        # bf16 matmul inputs run the PE at 1 cycle/row; bf16 tanh in/out
        # doubles ACT throughput. Values are in [-1,1] post-tanh and the
        # pre-tanh rounding is ~4e-3 absolute — both far inside tolerance.
        w_r = singles.tile([P, NT], bf16)
        nc.vector.tensor_copy(w_r[:], w_sb[:])

        # DRAM views with the H-halves split out: [b, p, t, n] so one DMA
        # pulls both halves of an n-chunk.
        st_r = st.rearrange("b (t p) n -> b p t n", p=P)
        dy_r = dy.rearrange("b (t p) n -> b p t n", p=P)

        # Uniform 2500-wide chunks; the last batch ends 1250, 625, 625 so
        # the post-final-load serial chain is short.
        work = []
        for b in range(B_LOC - 1):
            work += [(b, j * NC, NC) for j in range(N // NC)]
        work += [(3, 0, 2500), (3, 2500, 2500), (3, 5000, 2500)]
        work += [(3, 7500, 1250), (3, 8750, 625), (3, 9375, 625)]

        # Engine dispatch is in-order per engine, so a dma_start must never
        # be queued behind compute or a not-yet-ready store: static loads
        # and (deferred) stores ride SP; dynamic loads ride the gpsimd
        # SWDGE queue (the Pool engine runs no compute); ACT and DVE run
        # pure compute. Stores are emitted two items late so their copies
        # are long done when SP reaches them — a store's dependencies must
        # not stall the next static load behind it in the SP queue.
        pending_stores = []
        for b, n0, ncw in work:
            if len(pending_stores) >= 2:
                out_ap, stage_t = pending_stores.pop(0)
                nc.sync.dma_start(out_ap, stage_t)
            # Fused load of both H-halves: SBUF [128, 2*ncw], half t in
            # columns [t*ncw, (t+1)*ncw).
            s_t = s_pool.tile([P, NT * ncw], f32, tag="s")
            nc.sync.dma_start(
                s_t[:].rearrange("p (t n) -> p t n", t=NT),
                st_r[b, :, :, n0 : n0 + ncw],
            )
            d_t = d_pool.tile([P, NT * ncw], f32, tag="d")
            nc.gpsimd.dma_start(
                d_t[:].rearrange("p (t n) -> p t n", t=NT),
                dy_r[b, :, :, n0 : n0 + ncw],
            )
            # Per-half adds so tanh(h0) overlaps add(h1) on the two
            # engines; tanh needs one call per half anyway (different
            # per-partition bias column).
            tanh_tiles = []
            for t in range(NT):
                hs = slice(t * ncw, (t + 1) * ncw)
                a_t = a_pool.tile([P, ncw], bf16, tag="a")
                nc.vector.tensor_add(a_t[:], s_t[:, hs], d_t[:, hs])
                t_t = t_pool.tile([P, ncw], bf16, tag="t")
                nc.scalar.activation(
                    t_t[:],
                    a_t[:],
                    mybir.ActivationFunctionType.Tanh,
                    bias=dec_sb[:, t * B_LOC + b : t * B_LOC + b + 1],
                )
                tanh_tiles.append(t_t)
            # One store per chunk from a double-buffered stage row, so a
            # chunk's copies never wait on the previous chunk's store-DMA
            # completion (stage WAR serialization).
            stage = stage_pool.tile([1, ncw], f32, tag="stage")
            nj = (ncw + JC - 1) // JC
            for j in range(nj):
                j0 = j * JC
                jw = min(JC, ncw - j0)
                jl = slice(j0, j0 + jw)
                pt = psum_pool.tile([1, JC], f32, tag="pt")
                nc.tensor.matmul(
                    pt[:1, :jw], w_r[:, 0:1], tanh_tiles[0][:, jl],
                    start=True, stop=False,
                )
                nc.tensor.matmul(
                    pt[:1, :jw], w_r[:, 1:2], tanh_tiles[1][:, jl],
                    start=False, stop=True,
                )
                # Single-lane PSUM->SBUF copies; alternate engines so
                # neither DVE nor ACT eats the whole cost.
                if j % 2 == 0:
                    nc.vector.tensor_copy(stage[:, jl], pt[:1, :jw])
                else:
                    nc.scalar.copy(stage[:, jl], pt[:1, :jw])
            pending_stores.append((out[b : b + 1, n0 : n0 + ncw], stage[:]))
        for out_ap, stage_t in pending_stores:
            nc.sync.dma_start(out_ap, stage_t)

    nc.compile()
    return nc


def _run(inputs, **spmd_kwargs):
    from concourse import bass_utils

    if "nc" not in _cache:
        _cache["nc"] = _build()
    nc = _cache["nc"]

    static_hidden = np.asarray(inputs["static_hidden"], dtype=np.float32)
    dynamic_hidden = np.asarray(inputs["dynamic_hidden"], dtype=np.float32)
    decoder_hidden = np.asarray(inputs["decoder_hidden"], dtype=np.float32)
    W = np.ascontiguousarray(np.asarray(inputs["W"], dtype=np.float32))

    in_maps = []
    for i in range(N_CORES):
        sl = slice(i * B_LOC, (i + 1) * B_LOC)
        in_maps.append(
            {
                "static_hidden": np.ascontiguousarray(static_hidden[sl]),
                "dynamic_hidden": np.ascontiguousarray(dynamic_hidden[sl]),
                "decoder_hidden": np.ascontiguousarray(decoder_hidden[sl]),
                "W": W,
            }
        )
    res = bass_utils.run_bass_kernel_spmd(
        nc, in_maps, core_ids=list(range(N_CORES)), **spmd_kwargs
    )
    out = np.concatenate([r["attns"] for r in res.results], axis=0)
    return out, res


def kernel(**inputs):
    out, _ = _run(inputs)
    return out


# revision 10
# speedup vs baseline: 1.0308x; 1.0308x over previous
"""Bass/Tile TRN2 kernel for nn_Attention_12489764897521.

attns[b, n] = sum_h W[0, h] * tanh(decoder[b, h] + static[b, h, n] + dynamic[b, h, n])

Full shapes: static/dynamic [32, 256, 10000] f32, decoder [32, 256] f32,
W [1, 256] f32 -> attns [32, 10000] f32.

Sharding: data-parallel over batch B across 8 cores (4 batches/core); W
replicated. The kernel is HBM-bandwidth-bound: 82 MB of input per core at
the ~360-410 GB/s/core HBM share (each NC pair splits one HBM stack), so
the streaming floor is ~200-230 us depending on device tenancy. Everything
else is overhead to hide:

  - prologue (~7 us, fixed): engine boot barriers + ucode table loads; the
    W/decoder preloads ride the gpsimd SWDGE queue so the first packets on
    the two HWDGE rings are big streaming loads.
  - streaming: static loads on the SP HWDGE ring, dynamic on the ACT ring;
    uniform 2500-wide n-chunks, each a fused 3D-AP load of both H-halves
    [128, 2*2500] (2.56 MB/transfer). Pools are 3-deep so the rings never
    wait on compute.
  - per chunk: DVE adds s+d per half -> bf16 tiles; ACT tanh (bf16 in/out,
    2x rate) with the decoder column as bias; PE accumulates the two
    128-long H-half contractions into psum [1, <=500] slices (bf16 matmul,
    1 cyc/row); single-lane psum->SBUF copies alternate DVE/ACT; one store
    per chunk from a double-buffered stage row.
  - tail: the last batch ends 1250, 625, 625 wide so the trailing serial
    add->tanh->matmul->copy->store chain after the final load is short.
"""

from contextlib import ExitStack

import numpy as np

B, H, N = 32, 256, 10000
N_CORES = 8
B_LOC = B // N_CORES  # 4 batches per core
P = 128
NT = H // P  # 2 H-halves
NC = 2500  # n-chunk width; each load fuses both H-halves -> [128, 2*NC]
JC = 500  # matmul free-dim chunk (<= 512, one PSUM bank)

_cache = {}


def _build():
    import concourse.bacc as bacc
    import concourse.mybir as mybir
    import concourse.tile as tile

    nc = bacc.Bacc(
        "TRN2", target_bir_lowering=False, debug=False, num_devices=N_CORES
    )
    st = nc.dram_tensor(
        "static_hidden", [B_LOC, H, N], mybir.dt.float32, kind="ExternalInput"
    ).ap()
    dy = nc.dram_tensor(
        "dynamic_hidden", [B_LOC, H, N], mybir.dt.float32, kind="ExternalInput"
    ).ap()
    dec = nc.dram_tensor(
        "decoder_hidden", [B_LOC, H], mybir.dt.float32, kind="ExternalInput"
    ).ap()
    w = nc.dram_tensor("W", [1, H], mybir.dt.float32, kind="ExternalInput").ap()
    out = nc.dram_tensor(
        "attns", [B_LOC, N], mybir.dt.float32, kind="ExternalOutput"
    ).ap()

    f32 = mybir.dt.float32
    bf16 = mybir.dt.bfloat16
    with tile.TileContext(nc) as tc, ExitStack() as ctx:
        singles = ctx.enter_context(tc.tile_pool(name="singles", bufs=1))
        s_pool = ctx.enter_context(tc.tile_pool(name="s", bufs=3))
        d_pool = ctx.enter_context(tc.tile_pool(name="d", bufs=3))
        a_pool = ctx.enter_context(tc.tile_pool(name="a", bufs=6))
        t_pool = ctx.enter_context(tc.tile_pool(name="t", bufs=6))
        stage_pool = ctx.enter_context(tc.tile_pool(name="stage", bufs=3))
        psum_pool = ctx.enter_context(
            tc.tile_pool(name="psum", bufs=8, space="PSUM")
        )

        # W as two [128, 1] columns (one per H-half), decoder as [128, 1]
        # bias columns indexed [t * B_LOC + b]. These preloads ride the
        # gpsimd SWDGE queue so the two HWDGE rings' first packets are the
        # first big streaming loads.
        w_sb = singles.tile([P, NT], f32)
        w_cols = w.rearrange("o (t p) -> t p o", p=P)
        for t in range(NT):
            nc.gpsimd.dma_start(w_sb[:, t : t + 1], w_cols[t])

        dec_sb = singles.tile([P, NT * B_LOC], f32)
        dec_r = dec.rearrange("b (t p) -> t p b", p=P)
        for t in range(NT):
            nc.gpsimd.dma_start(dec_sb[:, t * B_LOC : (t + 1) * B_LOC], dec_r[t])

        # bf16 matmul inputs run the PE at 1 cycle/row; bf16 tanh in/out
        # doubles ACT throughput. Values are in [-1,1] post-tanh and the
        # pre-tanh rounding is ~4e-3 absolute — both far inside tolerance.
        w_r = singles.tile([P, NT], bf16)
        nc.vector.tensor_copy(w_r[:], w_sb[:])

        # DRAM views with the H-halves split out: [b, p, t, n] so one DMA
        # pulls both halves of an n-chunk.
        st_r = st.rearrange("b (t p) n -> b p t n", p=P)
        dy_r = dy.rearrange("b (t p) n -> b p t n", p=P)

        # Uniform 2500-wide chunks; the last batch ends 1250, 625, 625 so
        # the post-final-load serial chain is short.
        work = []
        for b in range(B_LOC - 1):
            work += [(b, j * NC, NC) for j in range(N // NC)]
        work += [(3, 0, 2500), (3, 2500, 2500), (3, 5000, 2500)]
        work += [(3, 7500, 1250), (3, 8750, 1250)]

        # Engine dispatch is in-order per engine, so a dma_start must never
        # be queued behind compute or a not-yet-ready store: static loads
        # and (deferred) stores ride SP; dynamic loads ride the gpsimd
        # SWDGE queue (the Pool engine runs no compute); ACT and DVE run
        # pure compute. Stores are emitted two items late so their copies
        # are long done when SP reaches them — a store's dependencies must
        # not stall the next static load behind it in the SP queue.
        pending_stores = []
        for b, n0, ncw in work:
            if len(pending_stores) >= 2:
                out_ap, stage_t = pending_stores.pop(0)
                nc.sync.dma_start(out_ap, stage_t)
            # Fused load of both H-halves: SBUF [128, 2*ncw], half t in
            # columns [t*ncw, (t+1)*ncw).
            s_t = s_pool.tile([P, NT * ncw], f32, tag="s")
            nc.sync.dma_start(
                s_t[:].rearrange("p (t n) -> p t n", t=NT),
                st_r[b, :, :, n0 : n0 + ncw],
            )
            d_t = d_pool.tile([P, NT * ncw], f32, tag="d")
            nc.gpsimd.dma_start(
                d_t[:].rearrange("p (t n) -> p t n", t=NT),
                dy_r[b, :, :, n0 : n0 + ncw],
            )
            # Per-half adds so tanh(h0) overlaps add(h1) on the two
            # engines; tanh needs one call per half anyway (different
            # per-partition bias column).
            tanh_tiles = []
            for t in range(NT):
                hs = slice(t * ncw, (t + 1) * ncw)
                a_t = a_pool.tile([P, ncw], bf16, tag="a")
                nc.vector.tensor_add(a_t[:], s_t[:, hs], d_t[:, hs])
                t_t = t_pool.tile([P, ncw], bf16, tag="t")
                nc.scalar.activation(
                    t_t[:],
                    a_t[:],
                    mybir.ActivationFunctionType.Tanh,
                    bias=dec_sb[:, t * B_LOC + b : t * B_LOC + b + 1],
                )
                tanh_tiles.append(t_t)
            # One store per chunk from a double-buffered stage row, so a
            # chunk's copies never wait on the previous chunk's store-DMA
            # completion (stage WAR serialization).
            stage = stage_pool.tile([1, ncw], f32, tag="stage")
            nj = (ncw + JC - 1) // JC
            for j in range(nj):
                j0 = j * JC
                jw = min(JC, ncw - j0)
                jl = slice(j0, j0 + jw)
                pt = psum_pool.tile([1, JC], f32, tag="pt")
                nc.tensor.matmul(
                    pt[:1, :jw], w_r[:, 0:1], tanh_tiles[0][:, jl],
                    start=True, stop=False,
                )
                nc.tensor.matmul(
                    pt[:1, :jw], w_r[:, 1:2], tanh_tiles[1][:, jl],
                    start=False, stop=True,
                )
                # Single-lane PSUM->SBUF copies; alternate engines so
                # neither DVE nor ACT eats the whole cost.
                if j % 2 == 0:
                    nc.vector.tensor_copy(stage[:, jl], pt[:1, :jw])
                else:
                    nc.scalar.copy(stage[:, jl], pt[:1, :jw])
            pending_stores.append((out[b : b + 1, n0 : n0 + ncw], stage[:]))
        for out_ap, stage_t in pending_stores:
            nc.sync.dma_start(out_ap, stage_t)

    nc.compile()
    return nc


def _run(inputs, **spmd_kwargs):
    from concourse import bass_utils

    if "nc" not in _cache:
        _cache["nc"] = _build()
    nc = _cache["nc"]

    static_hidden = np.asarray(inputs["static_hidden"], dtype=np.float32)
    dynamic_hidden = np.asarray(inputs["dynamic_hidden"], dtype=np.float32)
    decoder_hidden = np.asarray(inputs["decoder_hidden"], dtype=np.float32)
    W = np.ascontiguousarray(np.asarray(inputs["W"], dtype=np.float32))

    in_maps = []
    for i in range(N_CORES):
        sl = slice(i * B_LOC, (i + 1) * B_LOC)
        in_maps.append(
            {
                "static_hidden": np.ascontiguousarray(static_hidden[sl]),
                "dynamic_hidden": np.ascontiguousarray(dynamic_hidden[sl]),
                "decoder_hidden": np.ascontiguousarray(decoder_hidden[sl]),
                "W": W,
            }
        )
    res = bass_utils.run_bass_kernel_spmd(
        nc, in_maps, core_ids=list(range(N_CORES)), **spmd_kwargs
    )
    out = np.concatenate([r["attns"] for r in res.results], axis=0)
    return out, res


def kernel(**inputs):
    out, _ = _run(inputs)
    return out
